# revision 11
# baseline (speedup 1.0000x reference)
"""Haar DWT (2x2 stride-2 block decomposition) on 8 Trainium2 NeuronCores.

Input x: (32, 3, 512, 512) f32. Outputs (ll, lh, hl, hh): each (32, 3, 256, 256).

Sharding: pure data parallel over the batch dim — 4 images per core, viewed as
12 channel images of 512x512 per core.

I/O in bf16: the host casts x to bf16 and pre-transposes each core's shard to
partition-major [128, 12ch, 4t, 512] with the columns of each row deinterleaved
into [even(256) | odd(256)] blocks, so every load/store DMA moves multi-KB
contiguous runs per partition and every matmul streams contiguous columns.
This halves HBM traffic vs f32 (12.6 MB/core total) — the roofline for this
memory-bound op; the 2e-2 rel-err budget dwarfs bf16's ~2^-9 rounding.

BOTH butterfly stages run on the TensorEngine via PSUM accumulation, so the
only non-PE compute is one tiny negation and plain PSUM->SBUF copies. Per
128-row tile t (PSUM bank, 512 f32):

  mm1: W x [E|O]        (start) -> bank = [rowfly(E) | rowfly(O)]
  mm2: W x O   -> cols 0:256   (acc) adds rowfly(O):  cols 0:256  = ll|lh
  mm3: W x -E  -> cols 256:512 (acc) adds rowfly(-E): cols 256:512 = hl|hh

W maps 128 image rows to 64 halved row-sums (partitions 0..63) and 64 halved
row-diffs (64..127); +-0.5 weights are exact in bf16 and accumulation is fp32
PSUM, so the only roundings are the bf16 input quantization and the final
f32->bf16 store convert. The -E operand is built by DVE tensor_scalar at 4x
rate before the matmuls.

PSUM tiles cover HALF a channel (2 banks, bufs=4) so the PE never waits long
for a release, and each half's exit is split tile-wise: ACT copies the even
128-row tile, DVE the odd one — each into its own SBUF tile stored by its own
DMA (host re-interleaves), so the engines never serialize on a shared
destination. Loads are issued on the SP HWDGE ring (nc.sync), stores on the
ACT HWDGE ring (nc.scalar): two independent descriptor streams.
"""

import sys

import numpy as np

if "/opt/trn_rl_repo" not in sys.path:
    sys.path.insert(0, "/opt/trn_rl_repo")

import ml_dtypes

from concourse import bacc, bass, mybir
from concourse import tile
from concourse.bass_utils import run_bass_kernel_spmd

N_CORES = 8
B, C, H, W = 32, 3, 512, 512
BPC = B // N_CORES  # images per core
NCH = BPC * C  # channel images per core (12)
P = 128  # SBUF partitions
NT = H // P  # 128-row tiles per channel (4)
HW_OUT = H // 2  # 256
CHUNK = 2  # channels per DMA (1 MB bf16 per transfer)
NCHUNK = NCH // CHUNK
NH = NT // 2  # half-channels per channel (2)

_CACHE = {}


def _butterfly_weights():
    """W[k, m]: m<64 -> 0.5*(row 2m + row 2m+1); m>=64 -> 0.5*(row 2m'+1 - row 2m')."""
    w = np.zeros((P, P), dtype=np.float32)
    for m in range(64):
        w[2 * m, m] = 0.5
        w[2 * m + 1, m] = 0.5
        w[2 * m, 64 + m] = -0.5
        w[2 * m + 1, 64 + m] = 0.5
    return w.astype(ml_dtypes.bfloat16)


def _build():
    nc = bacc.Bacc("TRN2", target_bir_lowering=False, debug=False)
    bf16 = mybir.dt.bfloat16
    f32 = mybir.dt.float32
    # x[p, ch, t, w]: row 128*t + p of channel ch; w axis = [even(256)|odd(256)]
    x = nc.dram_tensor("x", [P, NCH, NT, W], bf16, kind="ExternalInput")
    w = nc.dram_tensor("w", [P, P], bf16, kind="ExternalInput")
    # outa holds even 128-row tiles (t=0,2), outb odd tiles (t=1,3); within a
    # tile: cols [0:256)=ll|lh (by partition half), [256:512)=hl|hh.
    outa = nc.dram_tensor("outa", [P, NCH, NH, W], bf16, kind="ExternalOutput")
    outb = nc.dram_tensor("outb", [P, NCH, NH, W], bf16, kind="ExternalOutput")
    xa = x.ap()
    oaa = outa.ap()
    oab = outb.ap()
    with tile.TileContext(nc) as tc:
        with (
            tc.tile_pool(name="p", bufs=4) as pool,
            tc.tile_pool(name="w", bufs=1) as wpool,
            tc.tile_pool(name="sc", bufs=3) as scpool,
            tc.tile_pool(name="ps", bufs=3, space=bass.MemorySpace.PSUM) as psum,
            tc.tile_pool(name="wm", bufs=1, space=bass.MemorySpace.PSUM) as wmpool,
        ):
            wt = wpool.tile([P, P], bf16)
            xin0 = pool.tile([P, CHUNK, NT, W], bf16)
            # first x load issued before the weight load: its data hits the
            # wire ~0.6us earlier and the weights are only needed by matmuls
            nc.sync.dma_start(out=xin0[:, 0], in_=xa[:, 0])
            nc.sync.dma_start(out=wt[:], in_=w.ap())
            nc.sync.dma_start(out=xin0[:, 1], in_=xa[:, 1])
            # warm the PE p-state: ~3us of continuous dummy matmuls on the
            # weight tile while the first data loads stream, so the real
            # matmuls start at full clock instead of the mid-ramp one
            warm = wmpool.tile([P, W], f32)
            for r in range(24):
                nc.tensor.matmul(warm[:, 0:P], wt[:], wt[:], start=True, stop=True)
            for i in range(NCHUNK):
                c0 = i * CHUNK
                if i == 0:
                    xin = xin0
                else:
                    xin = pool.tile([P, CHUNK, NT, W], bf16)
                    if i == NCHUNK - 1:
                        # split the last load so the pipeline drains faster
                        for c in range(CHUNK):
                            nc.sync.dma_start(out=xin[:, c], in_=xa[:, c0 + c])
                    else:
                        nc.sync.dma_start(out=xin[:], in_=xa[:, c0 : c0 + CHUNK])
                oA = pool.tile([P, CHUNK, NH, W], bf16)
                oB = pool.tile([P, CHUNK, NH, W], bf16)
                xv = xin[:].rearrange("p c t (eo j) -> p c t eo j", eo=2)
                for c in range(CHUNK):
                    # -E scratch (DVE tensor_scalar, bf16 4x rate); only needs
                    # xin, so DVE runs it while PE is on the previous half
                    scn = scpool.tile([P, NT, HW_OUT], bf16)
                    nc.vector.tensor_scalar_mul(scn[:], xv[:, c, :, 0, :], -1.0)
                    for h in range(NH):
                        pt = psum.tile([P, 2, W], f32)
                        for tl in range(2):
                            t = 2 * h + tl
                            nc.tensor.matmul(
                                pt[:, tl, :],
                                wt[:],
                                xin[:, c, t, :],
                                start=True,
                                stop=False,
                            )
                        for tl in range(2):
                            t = 2 * h + tl
                            nc.tensor.matmul(
                                pt[:, tl, 0:HW_OUT],
                                wt[:],
                                xv[:, c, t, 1, :],
                                start=False,
                                stop=False,
                            )
                        for tl in range(2):
                            t = 2 * h + tl
                            nc.tensor.matmul(
                                pt[:, tl, HW_OUT:W],
                                wt[:],
                                scn[:, t, :],
                                start=False,
                                stop=True,
                            )
                        # exit: ACT takes the even tile, DVE the odd tile
                        nc.scalar.copy(oA[:, c, h], pt[:, 0, :])
                        nc.vector.tensor_copy(oB[:, c, h], pt[:, 1, :])
                # oA stores ride the ACT HWDGE ring, oB stores the SP ring:
                # halves the per-engine issue cost and keeps both descriptor
                # streams fed. First/last chunks store per channel so the
                # output stream starts sooner / drains in smaller quanta.
                if i == 0 or i == NCHUNK - 1:
                    for c in range(CHUNK):
                        nc.scalar.dma_start(out=oaa[:, c0 + c], in_=oA[:, c])
                        nc.sync.dma_start(out=oab[:, c0 + c], in_=oB[:, c])
                else:
                    nc.scalar.dma_start(out=oaa[:, c0 : c0 + CHUNK], in_=oA[:])
                    nc.sync.dma_start(out=oab[:, c0 : c0 + CHUNK], in_=oB[:])
    nc.compile()
    return nc


def _get_nc():
    if "nc" not in _CACHE:
        _CACHE["nc"] = _build()
    return _CACHE["nc"]


def run(x, **spmd_kwargs):
    """Run the DWT on 8 cores; returns (results_tuple, BassKernelResults)."""
    nc = _get_nc()
    xbf = np.ascontiguousarray(np.asarray(x, dtype=np.float32)).astype(
        ml_dtypes.bfloat16
    )
    # (B,C,H,W) -> [core, NCH, NT, P, j, eo] -> [core, P, NCH, NT, eo, j]
    xs = xbf.reshape(N_CORES, NCH, NT, P, HW_OUT, 2).transpose(0, 3, 1, 2, 5, 4)
    xs = np.ascontiguousarray(xs).reshape(N_CORES, P, NCH, NT, W)
    wmat = _butterfly_weights()
    in_maps = [{"x": xs[i], "w": wmat} for i in range(N_CORES)]
    res = None
    for attempt in range(3):
        try:
            res = run_bass_kernel_spmd(
                nc, in_maps, core_ids=list(range(N_CORES)), **spmd_kwargs
            )
            break
        except Exception:
            # transient device wedge (NRT_EXEC_UNIT_UNRECOVERABLE) recovers
            # on retry; re-raise only if it persists
            if attempt == 2:
                raise
            import time

            time.sleep(2)
    # re-interleave tiles: outa has t=0,2; outb has t=1,3
    fa = np.stack([res.results[i]["outa"] for i in range(N_CORES)])
    fb = np.stack([res.results[i]["outb"] for i in range(N_CORES)])
    full = np.stack([fa, fb], axis=3)  # [core, P, NCH, tl(2), NH, W] -> t=2*h+tl
    # t = 2*h + tl with axes (tl, h): want [core, P, NCH, h, tl, W] -> flat t
    full = full.transpose(0, 1, 2, 4, 3, 5).reshape(N_CORES, P, NCH, NT, 2, HW_OUT)
    # -> (cores, NCH, NT, P, 2, j); out image row r = 64*t + (p mod 64)
    full = full.transpose(0, 2, 3, 1, 4, 5)
    full = np.ascontiguousarray(full).astype(np.float32)
    def expand(sl):  # (cores, NCH, NT, 64, j) -> (B, C, 256, 256)
        return np.ascontiguousarray(sl).reshape(B, C, HW_OUT, HW_OUT)
    ll = expand(full[:, :, :, 0:64, 0, :])
    lh = expand(full[:, :, :, 64:128, 0, :])
    hl = expand(full[:, :, :, 0:64, 1, :])
    hh = expand(full[:, :, :, 64:128, 1, :])
    return (ll, lh, hl, hh), res


def kernel(x):
    out, _ = run(x)
    return out


# revision 14
# speedup vs baseline: 1.0971x; 1.0971x over previous
"""Haar DWT (2x2 stride-2 block decomposition) on 8 Trainium2 NeuronCores.

Input x: (32, 3, 512, 512) f32. Outputs (ll, lh, hl, hh): each (32, 3, 256, 256).

Sharding: pure data parallel over the batch dim — 4 images per core, viewed as
12 channel images of 512x512 per core.

I/O in bf16: the host casts x to bf16 and pre-transposes each core's shard to
partition-major [128, 12ch, 4t, 512] with the columns of each row deinterleaved
into [even(256) | odd(256)] blocks, so every load/store DMA moves multi-KB
contiguous runs per partition and every matmul streams contiguous columns.
This halves HBM traffic vs f32 (12.6 MB/core total) — the roofline for this
memory-bound op; the 2e-2 rel-err budget dwarfs bf16's ~2^-9 rounding.

BOTH butterfly stages run on the TensorEngine via PSUM accumulation, so the
only non-PE compute is one tiny negation and plain PSUM->SBUF copies. Per
128-row tile t (PSUM bank, 512 f32):

  mm1: W x [E|O]        (start) -> bank = [rowfly(E) | rowfly(O)]
  mm2: W x O   -> cols 0:256   (acc) adds rowfly(O):  cols 0:256  = ll|lh
  mm3: W x -E  -> cols 256:512 (acc) adds rowfly(-E): cols 256:512 = hl|hh

W maps 128 image rows to 64 halved row-sums (partitions 0..63) and 64 halved
row-diffs (64..127); +-0.5 weights are exact in bf16 and accumulation is fp32
PSUM, so the only roundings are the bf16 input quantization and the final
f32->bf16 store convert. The -E operand is built by DVE tensor_scalar at 4x
rate before the matmuls.

PSUM tiles cover HALF a channel (2 banks, bufs=4) so the PE never waits long
for a release, and each half's exit is split tile-wise: ACT copies the even
128-row tile, DVE the odd one — each into its own SBUF tile stored by its own
DMA (host re-interleaves), so the engines never serialize on a shared
destination. Loads are issued on the SP HWDGE ring (nc.sync), stores on the
ACT HWDGE ring (nc.scalar): two independent descriptor streams.
"""

import sys

import numpy as np

if "/opt/trn_rl_repo" not in sys.path:
    sys.path.insert(0, "/opt/trn_rl_repo")

import ml_dtypes

from concourse import bacc, bass, mybir
from concourse import tile
from concourse.bass_utils import run_bass_kernel_spmd

N_CORES = 8
B, C, H, W = 32, 3, 512, 512
BPC = B // N_CORES  # images per core
NCH = BPC * C  # channel images per core (12)
P = 128  # SBUF partitions
NT = H // P  # 128-row tiles per channel (4)
HW_OUT = H // 2  # 256
CHUNK = 2  # channels per DMA (1 MB bf16 per transfer)
NCHUNK = NCH // CHUNK
NH = NT // 2  # half-channels per channel (2)

_CACHE = {}


def _butterfly_weights():
    """W[k, m]: m<64 -> 0.5*(row 2m + row 2m+1); m>=64 -> 0.5*(row 2m'+1 - row 2m')."""
    w = np.zeros((P, P), dtype=np.float32)
    for m in range(64):
        w[2 * m, m] = 0.5
        w[2 * m + 1, m] = 0.5
        w[2 * m, 64 + m] = -0.5
        w[2 * m + 1, 64 + m] = 0.5
    return w.astype(ml_dtypes.bfloat16)


def _build():
    nc = bacc.Bacc("TRN2", target_bir_lowering=False, debug=False)
    bf16 = mybir.dt.bfloat16
    f32 = mybir.dt.float32
    # x[p, ch, t, w]: row 128*t + p of channel ch; w axis = [even(256)|odd(256)]
    x = nc.dram_tensor("x", [P, NCH, NT, W], bf16, kind="ExternalInput")
    w = nc.dram_tensor("w", [P, P], bf16, kind="ExternalInput")
    # outa holds even 128-row tiles (t=0,2), outb odd tiles (t=1,3); within a
    # tile: cols [0:256)=ll|lh (by partition half), [256:512)=hl|hh.
    outa = nc.dram_tensor("outa", [P, NCH, NH, W], bf16, kind="ExternalOutput")
    outb = nc.dram_tensor("outb", [P, NCH, NH, W], bf16, kind="ExternalOutput")
    xa = x.ap()
    oaa = outa.ap()
    oab = outb.ap()
    with tile.TileContext(nc) as tc:
        with (
            tc.tile_pool(name="p", bufs=4) as pool,
            tc.tile_pool(name="w", bufs=1) as wpool,
            tc.tile_pool(name="sc", bufs=3) as scpool,
            tc.tile_pool(name="ps", bufs=4, space=bass.MemorySpace.PSUM) as psum,
        ):
            wt = wpool.tile([P, P], bf16)
            xin0 = pool.tile([P, CHUNK, NT, W], bf16)
            # first x load issued before the weight load: its data hits the
            # wire ~0.6us earlier and the weights are only needed by matmuls
            nc.sync.dma_start(out=xin0[:, 0], in_=xa[:, 0])
            nc.sync.dma_start(out=wt[:], in_=w.ap())
            nc.sync.dma_start(out=xin0[:, 1], in_=xa[:, 1])
            for i in range(NCHUNK):
                c0 = i * CHUNK
                if i == 0:
                    xin = xin0
                else:
                    xin = pool.tile([P, CHUNK, NT, W], bf16)
                    if i == NCHUNK - 1:
                        # split the last load so the pipeline drains faster
                        for c in range(CHUNK):
                            nc.sync.dma_start(out=xin[:, c], in_=xa[:, c0 + c])
                    else:
                        nc.sync.dma_start(out=xin[:], in_=xa[:, c0 : c0 + CHUNK])
                oA = pool.tile([P, CHUNK, NH, W], bf16)
                oB = pool.tile([P, CHUNK, NH, W], bf16)
                xv = xin[:].rearrange("p c t (eo j) -> p c t eo j", eo=2)
                for c in range(CHUNK):
                    # -E scratch (DVE tensor_scalar, bf16 4x rate); only needs
                    # xin, so DVE runs it while PE is on the previous half
                    scn = scpool.tile([P, NT, HW_OUT], bf16)
                    nc.vector.tensor_scalar_mul(scn[:], xv[:, c, :, 0, :], -1.0)
                    for h in range(NH):
                        pt = psum.tile([P, 2, W], f32)
                        for tl in range(2):
                            t = 2 * h + tl
                            nc.tensor.matmul(
                                pt[:, tl, :],
                                wt[:],
                                xin[:, c, t, :],
                                start=True,
                                stop=False,
                            )
                        for tl in range(2):
                            t = 2 * h + tl
                            nc.tensor.matmul(
                                pt[:, tl, 0:HW_OUT],
                                wt[:],
                                xv[:, c, t, 1, :],
                                start=False,
                                stop=False,
                            )
                        for tl in range(2):
                            t = 2 * h + tl
                            nc.tensor.matmul(
                                pt[:, tl, HW_OUT:W],
                                wt[:],
                                scn[:, t, :],
                                start=False,
                                stop=True,
                            )
                        # exit: ACT takes the even tile, DVE the odd tile
                        nc.scalar.copy(oA[:, c, h], pt[:, 0, :])
                        nc.vector.tensor_copy(oB[:, c, h], pt[:, 1, :])
                # first/last chunks store per channel so the output stream
                # starts sooner / drains in smaller quanta; all stores stay on
                # the ACT HWDGE ring (the SP ring's FIFO is busy with loads)
                if i == 0 or i == NCHUNK - 1:
                    for c in range(CHUNK):
                        nc.scalar.dma_start(out=oaa[:, c0 + c], in_=oA[:, c])
                        nc.scalar.dma_start(out=oab[:, c0 + c], in_=oB[:, c])
                else:
                    nc.scalar.dma_start(out=oaa[:, c0 : c0 + CHUNK], in_=oA[:])
                    nc.scalar.dma_start(out=oab[:, c0 : c0 + CHUNK], in_=oB[:])
    nc.compile()
    return nc


def _get_nc():
    if "nc" not in _CACHE:
        _CACHE["nc"] = _build()
    return _CACHE["nc"]


def run(x, **spmd_kwargs):
    """Run the DWT on 8 cores; returns (results_tuple, BassKernelResults)."""
    nc = _get_nc()
    xbf = np.ascontiguousarray(np.asarray(x, dtype=np.float32)).astype(
        ml_dtypes.bfloat16
    )
    # (B,C,H,W) -> [core, NCH, NT, P, j, eo] -> [core, P, NCH, NT, eo, j]
    xs = xbf.reshape(N_CORES, NCH, NT, P, HW_OUT, 2).transpose(0, 3, 1, 2, 5, 4)
    xs = np.ascontiguousarray(xs).reshape(N_CORES, P, NCH, NT, W)
    wmat = _butterfly_weights()
    in_maps = [{"x": xs[i], "w": wmat} for i in range(N_CORES)]
    res = None
    for attempt in range(3):
        try:
            res = run_bass_kernel_spmd(
                nc, in_maps, core_ids=list(range(N_CORES)), **spmd_kwargs
            )
            break
        except Exception:
            # transient device wedge (NRT_EXEC_UNIT_UNRECOVERABLE) recovers
            # on retry; re-raise only if it persists
            if attempt == 2:
                raise
            import time

            time.sleep(2)
    # re-interleave tiles: outa has t=0,2; outb has t=1,3
    fa = np.stack([res.results[i]["outa"] for i in range(N_CORES)])
    fb = np.stack([res.results[i]["outb"] for i in range(N_CORES)])
    full = np.stack([fa, fb], axis=3)  # [core, P, NCH, tl(2), NH, W] -> t=2*h+tl
    # t = 2*h + tl with axes (tl, h): want [core, P, NCH, h, tl, W] -> flat t
    full = full.transpose(0, 1, 2, 4, 3, 5).reshape(N_CORES, P, NCH, NT, 2, HW_OUT)
    # -> (cores, NCH, NT, P, 2, j); out image row r = 64*t + (p mod 64)
    full = full.transpose(0, 2, 3, 1, 4, 5)
    full = np.ascontiguousarray(full).astype(np.float32)
    def expand(sl):  # (cores, NCH, NT, 64, j) -> (B, C, 256, 256)
        return np.ascontiguousarray(sl).reshape(B, C, HW_OUT, HW_OUT)
    ll = expand(full[:, :, :, 0:64, 0, :])
    lh = expand(full[:, :, :, 64:128, 0, :])
    hl = expand(full[:, :, :, 0:64, 1, :])
    hh = expand(full[:, :, :, 64:128, 1, :])
    return (ll, lh, hl, hh), res


def kernel(x):
    out, _ = run(x)
    return out
